# revision 41
# baseline (speedup 1.0000x reference)
"""Decode-step attention-partition kernel for 8 Trainium2 NeuronCores.

Shape (hardcoded from the problem spec):
  x[16,1,4096], ln_w[4096], Wq/Wk/Wv/Wo[4096,4096],
  K_cache/V_cache[16,2048,32,128], cache_lens[16] int32.

Sharding: head-parallel. Core c owns heads [4c, 4c+4) for ALL 16 requests:
  - RMSNorm of x replicated (tiny).
  - q/k computed directly in transposed form qT/kT [128d, 16b] per head;
    v in natural form [16, 512].
  - Ragged attention over the per-request cache with the new token folded in
    at position L_b (kT column spliced into the transposed K tile; v row
    spliced into the V tile). Softmax without max-subtraction (scores are
    O(10) here so exp() is safe in fp32); normalization 1/Z folded into the
    per-head o_proj epilogue as a per-partition scale.
  - Each core emits a partial o_proj [16, 4096] (its 4 heads' contribution);
    the host sums the 8 partials and adds the residual.

Request lengths are read on the host and baked into the instruction stream
(static trip counts, exact-size DMAs). Requests are sorted by length
descending so per-t-block "valid request" sets are prefixes.
"""

import sys
import types
import ctypes
import contextlib

import numpy as np
import ml_dtypes

BF16_NP = ml_dtypes.bfloat16

# ---------------------------------------------------------------------------
# axon NTFF profile hook (the image's antenv lacks axon_hooks; the capability
# exists in libaxon_pjrt.so). Registered before concourse.bass_utils import.
# ---------------------------------------------------------------------------


def _install_ntff_hook():
    if "antenv.axon_hooks" in sys.modules:
        return
    try:
        lib = ctypes.CDLL("/opt/axon/libaxon_pjrt.so")
        lib.axon_start_nrt_profile.argtypes = [
            ctypes.POINTER(ctypes.c_int64),
            ctypes.c_size_t,
        ]
        lib.axon_start_nrt_profile.restype = ctypes.c_int64
        lib.axon_stop_nrt_profile.argtypes = [ctypes.c_char_p]
        lib.axon_stop_nrt_profile.restype = ctypes.c_int64
    except OSError:
        lib = None

    @contextlib.contextmanager
    def _hook(output_dir, device_ids):
        import jax

        jax.devices()
        if device_ids:
            ids = (ctypes.c_int64 * len(device_ids))(*device_ids)
            rc = lib.axon_start_nrt_profile(ids, len(device_ids))
        else:
            rc = lib.axon_start_nrt_profile(None, 0)
        if rc != 0:
            raise RuntimeError(f"axon_start_nrt_profile rc={rc}")
        try:
            yield
        finally:
            n = lib.axon_stop_nrt_profile(str(output_dir).encode())
            print(f"ntff profile: {n} file(s) -> {output_dir}", file=sys.stderr)

    mod = types.ModuleType("antenv.axon_hooks")
    mod.get_axon_ntff_profile_hook = (lambda: _hook) if lib is not None else (lambda: None)
    mod.set_axon_ntff_profile_hook = lambda h: None
    sys.modules["antenv.axon_hooks"] = mod


_install_ntff_hook()

import concourse.bass as bass
import concourse.mybir as mybir
import concourse.tile as tile
from concourse.vector_clock import ScopedClock
from concourse.masks import make_identity
from concourse.bass_utils import run_bass_kernel_spmd

# ---------------------------------------------------------------------------
# This walrus build rejects instructions with >1 semaphore wait command
# ("Too many sync wait commands" in setupSyncWait for CTRL structs). Tile's
# kernel-tail drain accumulates one wait per engine/DMA lane. Split the waits
# across preceding same-engine NOPs (1 wait each).
# ---------------------------------------------------------------------------
_MAXW = 1


def _patched_drain_and_barrier(self, tick_clock, wait_clock):
    nc = self.nc
    probe = nc.sync.nop(nofuse=True)
    wait_clock.add_sem_waits(probe.ins, ScopedClock({None: tick_clock.global_clock}))
    si = probe.ins.sync_info
    waits = list(si.on_wait) if si is not None else []
    if len(waits) > _MAXW:
        si.on_wait = waits[:_MAXW]
        for i in range(_MAXW, len(waits), _MAXW):
            nop = nc.sync.nop(nofuse=True)
            nop.ins.sync_info = mybir.SyncInfo(
                on_wait=waits[i : i + _MAXW], on_update=[]
            )
    nc.sync.drain()
    nc.all_engine_barrier()
    assert self.sems is not None
    popped = nc._tile_sem_poison_stack.pop()
    assert popped is self._sem_poison
    nc.clear_and_free_semaphores(list(self.sems.allocated().values()))
    nc.all_engine_barrier()


tile.TileContext._drain_and_barrier = _patched_drain_and_barrier

_wsplit_counter = [0]


def _split_excess_waits(nc):
    """Post-pass: this walrus build allows at most 1 sem-wait per instruction.
    Move excess waits onto preceding same-engine NoOps (same-engine program
    order preserves the wait semantics)."""
    for fn in nc.m.functions:
        for bb in fn.blocks:
            out = []
            changed = False
            for inst in bb.instructions:
                si = inst.sync_info
                if (
                    si is not None
                    and len(si.on_wait) > 1
                    and not isinstance(inst, mybir.InstAllEngineBarrier)
                ):
                    waits = list(si.on_wait)
                    for w in waits[:-1]:
                        _wsplit_counter[0] += 1
                        out.append(
                            mybir.InstNoOp(
                                name=f"I-wsplit-{_wsplit_counter[0]}",
                                engine=inst.engine,
                                sync_info=mybir.SyncInfo(
                                    on_wait=[w], on_update=[]
                                ),
                            )
                        )
                    si.on_wait = [waits[-1]]
                    changed = True
                out.append(inst)
            if changed:
                bb.instructions[:] = out

# ---------------------------------------------------------------------------

F32 = mybir.dt.float32
BF16 = mybir.dt.bfloat16
P = 128
B = 16
T = 2048
D = 4096
H = 32
HD = 128
NHL = 4          # heads per core
NCORES = 8
EPS = 1e-6
NKC = D // P     # 32 contraction chunks for the projections
SCALE = 1.0 / float(np.sqrt(HD))
JG = 16          # t-blocks per V DMA group (one transfer per request)


def _build(Ls):
    """Build the per-core Bass kernel. Ls: 16 request lengths, sorted desc."""
    nblk = [l // P + 1 for l in Ls]          # t-blocks incl. the new token
    r = [l % P for l in Ls]                  # new-token row within tail block
    vt = [rr + 1 for rr in r]                # valid rows in tail block
    jmax = max(nblk)
    # tile j is touched by requests [0, nbj[j]) (lengths sorted descending)
    nbj = [sum(1 for b in range(B) if nblk[b] > j) for j in range(jmax)]

    nc = bass.Bass()
    x_d = nc.dram_tensor("x", [B, D], F32, kind="ExternalInput")
    lnw_d = nc.dram_tensor("lnw", [P, NKC], F32, kind="ExternalInput")
    wq_d = nc.dram_tensor("wq", [D, NHL * HD], BF16, kind="ExternalInput")
    wk_d = nc.dram_tensor("wk", [D, NHL * HD], BF16, kind="ExternalInput")
    wv_d = nc.dram_tensor("wv", [D, NHL * HD], BF16, kind="ExternalInput")
    wo_d = nc.dram_tensor("wo", [NHL * HD, D], BF16, kind="ExternalInput")
    ktc_d = nc.dram_tensor("ktc", [B, NHL, HD, T], BF16, kind="ExternalInput")
    vc_d = nc.dram_tensor("vc", [B, T, NHL * HD], BF16, kind="ExternalInput")
    out_d = nc.dram_tensor("out", [B, D], F32, kind="ExternalOutput")

    with tile.TileContext(nc) as tc:
        with (
            tc.tile_pool(name="const", bufs=1) as const_pool,
            tc.tile_pool(name="persist", bufs=1) as persist,
            tc.tile_pool(name="pcols", bufs=1) as p_pool,
        ):
            identity = const_pool.tile([P, P], F32, tag="identity")
            make_identity(nc, identity[:])
            ones_col = const_pool.tile([P, 1], BF16, tag="ones")
            nc.gpsimd.memset(ones_col[:], 1.0)
            identity_bf = const_pool.tile([P, P], BF16, tag="idbf")
            make_identity(nc, identity_bf[:])
            zrow = const_pool.tile([1, 512], F32, tag="zrow")
            nc.gpsimd.memset(zrow[:], 0.0)
            lnw_sb = const_pool.tile([P, NKC], F32, tag="lnw")
            nc.sync.dma_start(lnw_sb[:], lnw_d[:, :])

            # ---------------- phase 1: RMSNorm + projections ----------------
            qt_sb = persist.tile([P, B * NHL], BF16, tag="qt")  # col h*16+b
            kt_sb = persist.tile([P, B * NHL], BF16, tag="kt")  # col h*16+b
            v_sb = persist.tile([B, NHL * HD], BF16, tag="vsb")
            xnt_sb = persist.tile([P, NKC * B], BF16, tag="xnt")  # chunk kc at cols kc*16

            # ---------------- score PSUM setup ----------------
            zatt_cm = tc.tile_pool(name="zatt", bufs=1, space="PSUM")
            zatt_pool = zatt_cm.__enter__()
            sc_cm = tc.tile_pool(name="sc", bufs=1, space="PSUM")
            sc_pool = sc_cm.__enter__()
            n_sc_banks = (jmax + 7) // 8
            sc_ps = [
                sc_pool.tile([P, 512], F32, tag=f"sc{i}", name=f"sc{i}")
                for i in range(n_sc_banks)
            ]
            z_ps = zatt_pool.tile([1, B * NHL], F32, tag="z")
            attn_ps = zatt_pool.tile([P, B * NHL], F32, tag="attn")

            def sc_slice(j, c0, c1, p0, p1):
                return sc_ps[j // 8][p0:p1, (j % 8) * 64 + c0 : (j % 8) * 64 + c1]

            # memset all score banks to -1e30 (invalid rows stay masked)
            for t_ in sc_ps:
                nc.vector.memset(t_[:], -1.0e30)
            nc.tensor.matmul(
                z_ps[:], zrow[0:1, 0:1], zrow[0:1, : B * NHL],
                start=True, stop=True, skip_group_check=True,
            )
            nc.tensor.matmul(
                attn_ps[:], zrow[0:1, :P], zrow[0:1, : B * NHL],
                start=True, stop=True, skip_group_check=True,
            )

            p1sb_cm = tc.tile_pool(name="p1sb", bufs=1)
            p1sb = p1sb_cm.__enter__()
            wpool_cm = tc.tile_pool(name="wpool", bufs=2)
            wpool = wpool_cm.__enter__()
            xntp_cm = tc.tile_pool(name="xntp", bufs=2, space="PSUM")
            xntp = xntp_cm.__enter__()
            qkv_cm = tc.tile_pool(name="qkvps", bufs=2, space="PSUM")
            qkvps = qkv_cm.__enter__()

            x_sb = p1sb.tile([B, D], F32, tag="x")
            nc.sync.dma_start(x_sb[:], x_d[:, :])
            # transpose RAW x (scaled by ln_w only); the 1/rms factor is a
            # per-request scalar folded into the q/k/v PSUM evacuations, so
            # the transposes don't wait on the reduction chain
            for kc in range(NKC):
                xnt_ps = xntp.tile([P, B], F32, tag="xntps")
                nc.tensor.transpose(
                    xnt_ps[:],
                    x_sb[:, kc * P : (kc + 1) * P],
                    identity[:B, :B],
                )
                nc.scalar.mul(
                    xnt_sb[:, kc * B : (kc + 1) * B], xnt_ps[:],
                    lnw_sb[:, kc : kc + 1],
                )
            xsq = p1sb.tile([B, D], F32, tag="xsq")
            ssq = p1sb.tile([B, 1], F32, tag="ssq")
            nc.scalar.activation(
                xsq[:], x_sb[:], mybir.ActivationFunctionType.Square,
                accum_out=ssq[:],
            )
            ssq2 = p1sb.tile([B, 1], F32, tag="ssq2")
            nc.vector.tensor_scalar_add(ssq2[:], ssq[:], EPS * D)
            std = p1sb.tile([B, 1], F32, tag="std")
            nc.scalar.activation(
                std[:], ssq2[:], mybir.ActivationFunctionType.Sqrt,
                scale=1.0 / D,
            )
            rstd = p1sb.tile([B, 1], F32, tag="rstd")
            nc.vector.reciprocal(rstd[:], std[:])

            # natural-form projections [16, 512]; each accumulator is alone in
            # its bank so plain start/stop group semantics are safe
            q_ps = qkvps.tile([B, NHL * HD], F32, tag="qkv", name="qps")
            WGRP = 8

            def w_load(pool_tag, dram):
                tiles = []
                for g in range(NKC // WGRP):
                    w_big = wpool.tile(
                        [P, WGRP * NHL * HD], BF16, tag=pool_tag,
                        name=f"{pool_tag}{g}",
                    )
                    srcw = dram[
                        g * WGRP * P : (g + 1) * WGRP * P, :
                    ].rearrange("(kc p) n -> p kc n", p=P)
                    nc.sync.dma_start(
                        w_big[:].rearrange("p (kc n) -> p kc n", n=NHL * HD),
                        srcw,
                    )
                    tiles.append(w_big)
                return tiles

            def proj(w_tiles, acc_ps):
                for kc in range(NKC):
                    wslice = w_tiles[kc // WGRP][
                        :, (kc % WGRP) * NHL * HD : (kc % WGRP + 1) * NHL * HD
                    ]
                    nc.tensor.matmul(
                        acc_ps[:], xnt_sb[:, kc * B : (kc + 1) * B], wslice,
                        start=(kc == 0), stop=(kc == NKC - 1),
                    )

            def transpose4(nat_sb, dst_sb):
                for h in range(NHL):
                    t_ps = xntp.tile([P, B], F32, tag="xntps", name=f"tp{h}")
                    nc.tensor.transpose(
                        t_ps[:], nat_sb[:, h * HD : (h + 1) * HD],
                        identity[:B, :B],
                    )
                    nc.scalar.copy(dst_sb[:, h * B : (h + 1) * B], t_ps[:])

            wq_tiles = w_load("wq", wq_d)
            proj(wq_tiles, q_ps)
            q_nat = p1sb.tile([B, NHL * HD], F32, tag="qnat")
            nc.scalar.mul(q_nat[:], q_ps[:], rstd[:])
            transpose4(q_nat, qt_sb)

            # ---------------- pass K main (full blocks; needs only qt) ------
            # K is host-pre-transposed: ktc[b, h, d, t]. One DMA per request
            # covers all 4 heads' full cache blocks; scores read lhsT slices
            # straight out of the tile (no PE transpose, no evacuation copy).
            TW = T
            with tc.tile_pool(name="ktbig", bufs=3) as ktb_pool:
                for b in range(B):
                    L = Ls[b]
                    f = L // P
                    if f > 0:
                        ktb = ktb_pool.tile([P, NHL * TW], BF16, tag="ktb")
                        nc.sync.dma_start(
                            ktb[:].rearrange("p (h t) -> p h t", t=TW)[
                                :, :, : f * P
                            ],
                            ktc_d[b, :, :, : f * P].rearrange("h d t -> d h t"),
                        )
                        for j in range(f):
                            for h in range(NHL):
                                col = b * NHL + h
                                nc.tensor.matmul(
                                    sc_slice(j, col, col + 1, 0, P),
                                    ktb[:, h * TW + j * P : h * TW + (j + 1) * P],
                                    qt_sb[:, h * B + b : h * B + b + 1],
                                    start=True, stop=True,
                                )

            # k projection (weights prefetched during the scores above)
            k_ps = qkvps.tile([B, NHL * HD], F32, tag="qkv", name="kps")
            wk_tiles = w_load("wk", wk_d)
            proj(wk_tiles, k_ps)
            k_nat = p1sb.tile([B, NHL * HD], F32, tag="knat")
            nc.scalar.mul(k_nat[:], k_ps[:], rstd[:])
            transpose4(k_nat, kt_sb)

            # ---------------- pass K tails (need kt for the new token) ------
            with tc.tile_pool(name="kttail", bufs=4) as ktt_pool:
                for b in range(B):
                    L = Ls[b]
                    jt = L // P
                    ktt = ktt_pool.tile([P, NHL * P], BF16, tag="ktt")
                    if r[b] > 0:
                        nc.sync.dma_start(
                            ktt[:].rearrange("p (h t) -> p h t", t=P)[
                                :, :, : r[b]
                            ],
                            ktc_d[
                                b, :, :, jt * P : jt * P + r[b]
                            ].rearrange("h d t -> d h t"),
                        )
                    for h in range(NHL):
                        nc.vector.tensor_copy(
                            ktt[:, h * P + r[b] : h * P + r[b] + 1],
                            kt_sb[:, h * B + b : h * B + b + 1],
                        )
                        col = b * NHL + h
                        nc.tensor.matmul(
                            sc_slice(jt, col, col + 1, 0, vt[b]),
                            ktt[:, h * P : h * P + vt[b]],
                            qt_sb[:, h * B + b : h * B + b + 1],
                            start=True, stop=True,
                        )

            # v projection
            v_ps = qkvps.tile([B, NHL * HD], F32, tag="qkv", name="vps")
            wv_tiles = w_load("wv", wv_d)
            proj(wv_tiles, v_ps)
            nc.scalar.mul(v_sb[:], v_ps[:], rstd[:])

            qkv_cm.__exit__(None, None, None)
            xntp_cm.__exit__(None, None, None)
            wpool_cm.__exit__(None, None, None)
            p1sb_cm.__exit__(None, None, None)

            # ---------------- softmax (no max-sub) ----------------
            p_cols = []
            for j in range(jmax):
                pc = p_pool.tile([P, B * NHL], BF16, tag=f"p{j}", name=f"p{j}")
                nc.scalar.activation(
                    pc[:, : NHL * nbj[j]],
                    sc_slice(j, 0, NHL * nbj[j], 0, P),
                    mybir.ActivationFunctionType.Exp,
                    scale=SCALE,
                )
                p_cols.append(pc)
            for j in range(jmax):
                nc.tensor.matmul(
                    z_ps[0:1, : NHL * nbj[j]],
                    ones_col[:],
                    p_cols[j][:, : NHL * nbj[j]],
                    start=False, stop=(j == jmax - 1),
                    skip_group_check=True,
                )
            invz_row = persist.tile([1, B * NHL], F32, tag="invzr")
            nc.vector.reciprocal(invz_row[:], z_ps[:])
            # bounce through DRAM, reordering (b,h) -> (h,b) on the second hop
            # so the result is a per-partition column matching attn's h*B+b
            # column order (free dims can't become partitions inside SBUF)
            invz_dram = nc.dram_tensor("invz_scratch", [1, B * NHL], F32)
            nc.gpsimd.dma_start(invz_dram[:, :], invz_row[:])
            invz_col = persist.tile([NHL * B, 1], F32, tag="invzc")
            for h in range(NHL):
                nc.gpsimd.dma_start(
                    invz_col[h * B : (h + 1) * B, :],
                    invz_dram.rearrange("o (b h) -> (o b) h", h=NHL)[:, h : h + 1],
                )

            # ---------------- pass V: attn = p @ V ----------------
            with tc.tile_pool(name="vpool", bufs=3) as vpool:
                for b in range(B):
                    L = Ls[b]
                    ngrp = (nblk[b] + JG - 1) // JG
                    for jg in range(ngrp):
                        rows_g = max(0, min(JG * P, L - jg * JG * P))
                        q128, rem = divmod(rows_g, P)
                        v_tile = vpool.tile([P, JG * NHL * HD], BF16, tag="v")
                        if q128 > 0:
                            src = vc_d[
                                b, jg * JG * P : jg * JG * P + q128 * P, :
                            ].rearrange("(jj p) d -> p jj d", p=P)
                            nc.sync.dma_start(
                                v_tile[:, : q128 * NHL * HD].rearrange(
                                    "p (jj d) -> p jj d", d=NHL * HD
                                ),
                                src,
                            )
                        if rem > 0:
                            nc.sync.dma_start(
                                v_tile[
                                    :rem, q128 * NHL * HD : (q128 + 1) * NHL * HD
                                ],
                                vc_d[
                                    b,
                                    jg * JG * P + q128 * P : jg * JG * P + rows_g,
                                    :,
                                ],
                            )
                        for jj in range(JG):
                            j = jg * JG + jj
                            if j >= nblk[b]:
                                break
                            tail = j == nblk[b] - 1
                            m = vt[b] if tail else P
                            if tail:
                                # splice the new token's v row in
                                nc.gpsimd.dma_start(
                                    v_tile[
                                        r[b] : r[b] + 1,
                                        jj * NHL * HD : (jj + 1) * NHL * HD,
                                    ],
                                    v_sb[b : b + 1, :],
                                )
                            for h in range(NHL):
                                col = b * NHL + h
                                # attn columns are head-major so o_proj's lhsT
                                # per head is a contiguous [128, 16] slice
                                nc.tensor.matmul(
                                    attn_ps[:, h * B + b : h * B + b + 1],
                                    v_tile[
                                        :m,
                                        jj * NHL * HD + h * HD : jj * NHL * HD + (h + 1) * HD,
                                    ],
                                    p_cols[j][:m, col : col + 1],
                                    start=False, stop=tail,
                                    skip_group_check=True,
                                )

            attn_sb = persist.tile([P, B * NHL], BF16, tag="attnsb")
            nc.scalar.copy(attn_sb[:], attn_ps[:])
            sc_cm.__exit__(None, None, None)

            # ---------------- o_proj partial with 1/Z ----------------
            # scale attn by 1/Z in its transposed domain, where (h,b) is the
            # partition index and 1/Z is a plain per-partition scalar; then
            # o_proj accumulates the 4 heads in PSUM (no per-head epilogue)
            out_sb = persist.tile([B, D], F32, tag="outsb")
            with (
                tc.tile_pool(name="wopool", bufs=1) as wopool,
                tc.tile_pool(name="ops", bufs=4, space="PSUM") as o_ps_pool,
            ):
                tr1 = o_ps_pool.tile([B * NHL, P], BF16, tag="tr1", bufs=1)
                nc.tensor.transpose(tr1[:], attn_sb[:], identity_bf[:])
                attn_n = persist.tile([B * NHL, P], BF16, tag="attnn")
                nc.scalar.mul(attn_n[:], tr1[:], invz_col[:])
                tr2 = o_ps_pool.tile([P, B * NHL], BF16, tag="tr2", bufs=1)
                nc.tensor.transpose(
                    tr2[:], attn_n[:], identity_bf[: B * NHL, : B * NHL]
                )
                attn_s = persist.tile([P, B * NHL], BF16, tag="attns")
                nc.vector.tensor_copy(attn_s[:], tr2[:])

                NCH = D // 512
                wo_tiles = []
                for h in range(NHL):
                    wo_sb = wopool.tile([P, D], BF16, tag=f"wo{h}", name=f"wo{h}")
                    nc.scalar.dma_start(wo_sb[:], wo_d[h * HD : (h + 1) * HD, :])
                    wo_tiles.append(wo_sb)
                for nch in range(NCH):
                    o_ps = o_ps_pool.tile([B, 512], F32, tag="ops", bufs=2)
                    for h in range(NHL):
                        nc.tensor.matmul(
                            o_ps[:],
                            attn_s[:, h * B : (h + 1) * B],
                            wo_tiles[h][:, nch * 512 : (nch + 1) * 512],
                            start=(h == 0), stop=(h == NHL - 1),
                        )
                    nc.scalar.copy(out_sb[:, nch * 512 : (nch + 1) * 512], o_ps[:])
            nc.sync.dma_start(out_d[:, :], out_sb[:])
            zatt_cm.__exit__(None, None, None)

    _split_excess_waits(nc)
    return nc


def _prep_inputs(x, ln_w, Wq, Wk, Wv, Wo, K_cache, V_cache, cache_lens):
    x = np.asarray(x, np.float32).reshape(B, D)
    ln_w = np.asarray(ln_w, np.float32)
    cache_lens = np.asarray(cache_lens, np.int32)
    perm = np.argsort(-cache_lens, kind="stable")
    Ls = [int(cache_lens[p]) for p in perm]
    lnw2d = np.ascontiguousarray(ln_w.reshape(NKC, P).T)
    x_s = np.ascontiguousarray(x[perm])
    K4 = np.asarray(K_cache, np.float32).reshape(B, T, H, HD)
    V4 = np.asarray(V_cache, np.float32).reshape(B, T, H, HD)
    in_maps = []
    for c in range(NCORES):
        h0 = c * NHL
        in_maps.append(
            {
                "x": x_s,
                "lnw": lnw2d,
                "wq": np.ascontiguousarray(
                    np.asarray(Wq, np.float32)[:, h0 * HD : (h0 + NHL) * HD]
                ).astype(BF16_NP),
                "wk": np.ascontiguousarray(
                    np.asarray(Wk, np.float32)[:, h0 * HD : (h0 + NHL) * HD]
                ).astype(BF16_NP),
                "wv": np.ascontiguousarray(
                    np.asarray(Wv, np.float32)[:, h0 * HD : (h0 + NHL) * HD]
                ).astype(BF16_NP),
                "wo": np.ascontiguousarray(
                    np.asarray(Wo, np.float32)[h0 * HD : (h0 + NHL) * HD, :]
                ).astype(BF16_NP),
                "ktc": np.ascontiguousarray(
                    K4[perm][:, :, h0 : h0 + NHL, :].transpose(0, 2, 3, 1)
                ).astype(BF16_NP),
                "vc": np.ascontiguousarray(
                    V4[perm][:, :, h0 : h0 + NHL, :]
                ).reshape(B, T, NHL * HD).astype(BF16_NP),
            }
        )
    return in_maps, Ls, perm, x_s


def _run(x, ln_w, Wq, Wk, Wv, Wo, K_cache, V_cache, cache_lens, trace=False):
    in_maps, Ls, perm, x_s = _prep_inputs(
        x, ln_w, Wq, Wk, Wv, Wo, K_cache, V_cache, cache_lens
    )
    nc = _build(Ls)
    # the axon-proxied runtime occasionally hits a transient
    # NRT_EXEC_UNIT_UNRECOVERABLE; retry a couple of times
    last_exc = None
    for _attempt in range(3):
        try:
            res = run_bass_kernel_spmd(
                nc, in_maps, core_ids=list(range(NCORES)), trace=trace
            )
            break
        except Exception as e:  # noqa: BLE001
            last_exc = e
            import time as _time

            _time.sleep(2.0)
    else:
        raise last_exc
    partial = np.zeros((B, D), np.float32)
    for c in range(NCORES):
        partial += res.results[c]["out"]
    out_sorted = x_s + partial
    out = np.empty((B, D), np.float32)
    out[perm] = out_sorted
    return out.reshape(B, 1, D), res


def kernel(x, ln_w, Wq, Wk, Wv, Wo, K_cache, V_cache, cache_lens):
    out, _ = _run(x, ln_w, Wq, Wk, Wv, Wo, K_cache, V_cache, cache_lens)
    return out


# revision 42
# speedup vs baseline: 1.0533x; 1.0533x over previous
"""Decode-step attention-partition kernel for 8 Trainium2 NeuronCores.

Shape (hardcoded from the problem spec):
  x[16,1,4096], ln_w[4096], Wq/Wk/Wv/Wo[4096,4096],
  K_cache/V_cache[16,2048,32,128], cache_lens[16] int32.

Sharding: head-parallel. Core c owns heads [4c, 4c+4) for ALL 16 requests:
  - RMSNorm of x replicated (tiny).
  - q/k computed directly in transposed form qT/kT [128d, 16b] per head;
    v in natural form [16, 512].
  - Ragged attention over the per-request cache with the new token folded in
    at position L_b (kT column spliced into the transposed K tile; v row
    spliced into the V tile). Softmax without max-subtraction (scores are
    O(10) here so exp() is safe in fp32); normalization 1/Z folded into the
    per-head o_proj epilogue as a per-partition scale.
  - Each core emits a partial o_proj [16, 4096] (its 4 heads' contribution);
    the host sums the 8 partials and adds the residual.

Request lengths are read on the host and baked into the instruction stream
(static trip counts, exact-size DMAs). Requests are sorted by length
descending so per-t-block "valid request" sets are prefixes.
"""

import sys
import types
import ctypes
import contextlib

import numpy as np
import ml_dtypes

BF16_NP = ml_dtypes.bfloat16

# ---------------------------------------------------------------------------
# axon NTFF profile hook (the image's antenv lacks axon_hooks; the capability
# exists in libaxon_pjrt.so). Registered before concourse.bass_utils import.
# ---------------------------------------------------------------------------


def _install_ntff_hook():
    if "antenv.axon_hooks" in sys.modules:
        return
    try:
        lib = ctypes.CDLL("/opt/axon/libaxon_pjrt.so")
        lib.axon_start_nrt_profile.argtypes = [
            ctypes.POINTER(ctypes.c_int64),
            ctypes.c_size_t,
        ]
        lib.axon_start_nrt_profile.restype = ctypes.c_int64
        lib.axon_stop_nrt_profile.argtypes = [ctypes.c_char_p]
        lib.axon_stop_nrt_profile.restype = ctypes.c_int64
    except OSError:
        lib = None

    @contextlib.contextmanager
    def _hook(output_dir, device_ids):
        import jax

        jax.devices()
        if device_ids:
            ids = (ctypes.c_int64 * len(device_ids))(*device_ids)
            rc = lib.axon_start_nrt_profile(ids, len(device_ids))
        else:
            rc = lib.axon_start_nrt_profile(None, 0)
        if rc != 0:
            raise RuntimeError(f"axon_start_nrt_profile rc={rc}")
        try:
            yield
        finally:
            n = lib.axon_stop_nrt_profile(str(output_dir).encode())
            print(f"ntff profile: {n} file(s) -> {output_dir}", file=sys.stderr)

    mod = types.ModuleType("antenv.axon_hooks")
    mod.get_axon_ntff_profile_hook = (lambda: _hook) if lib is not None else (lambda: None)
    mod.set_axon_ntff_profile_hook = lambda h: None
    sys.modules["antenv.axon_hooks"] = mod


_install_ntff_hook()

import concourse.bass as bass
import concourse.mybir as mybir
import concourse.tile as tile
from concourse.vector_clock import ScopedClock
from concourse.masks import make_identity
from concourse.bass_utils import run_bass_kernel_spmd

# ---------------------------------------------------------------------------
# This walrus build rejects instructions with >1 semaphore wait command
# ("Too many sync wait commands" in setupSyncWait for CTRL structs). Tile's
# kernel-tail drain accumulates one wait per engine/DMA lane. Split the waits
# across preceding same-engine NOPs (1 wait each).
# ---------------------------------------------------------------------------
_MAXW = 1


def _patched_drain_and_barrier(self, tick_clock, wait_clock):
    nc = self.nc
    probe = nc.sync.nop(nofuse=True)
    wait_clock.add_sem_waits(probe.ins, ScopedClock({None: tick_clock.global_clock}))
    si = probe.ins.sync_info
    waits = list(si.on_wait) if si is not None else []
    if len(waits) > _MAXW:
        si.on_wait = waits[:_MAXW]
        for i in range(_MAXW, len(waits), _MAXW):
            nop = nc.sync.nop(nofuse=True)
            nop.ins.sync_info = mybir.SyncInfo(
                on_wait=waits[i : i + _MAXW], on_update=[]
            )
    nc.sync.drain()
    nc.all_engine_barrier()
    assert self.sems is not None
    popped = nc._tile_sem_poison_stack.pop()
    assert popped is self._sem_poison
    nc.clear_and_free_semaphores(list(self.sems.allocated().values()))
    nc.all_engine_barrier()


tile.TileContext._drain_and_barrier = _patched_drain_and_barrier

_wsplit_counter = [0]


def _split_excess_waits(nc):
    """Post-pass: this walrus build allows at most 1 sem-wait per instruction.
    Move excess waits onto preceding same-engine NoOps (same-engine program
    order preserves the wait semantics)."""
    for fn in nc.m.functions:
        for bb in fn.blocks:
            out = []
            changed = False
            for inst in bb.instructions:
                si = inst.sync_info
                if (
                    si is not None
                    and len(si.on_wait) > 1
                    and not isinstance(inst, mybir.InstAllEngineBarrier)
                ):
                    waits = list(si.on_wait)
                    for w in waits[:-1]:
                        _wsplit_counter[0] += 1
                        out.append(
                            mybir.InstNoOp(
                                name=f"I-wsplit-{_wsplit_counter[0]}",
                                engine=inst.engine,
                                sync_info=mybir.SyncInfo(
                                    on_wait=[w], on_update=[]
                                ),
                            )
                        )
                    si.on_wait = [waits[-1]]
                    changed = True
                out.append(inst)
            if changed:
                bb.instructions[:] = out

# ---------------------------------------------------------------------------

F32 = mybir.dt.float32
BF16 = mybir.dt.bfloat16
P = 128
B = 16
T = 2048
D = 4096
H = 32
HD = 128
NHL = 4          # heads per core
NCORES = 8
EPS = 1e-6
NKC = D // P     # 32 contraction chunks for the projections
SCALE = 1.0 / float(np.sqrt(HD))
JG = 8           # t-blocks per K/V DMA group (8*128 rows x 1KB bf16 = 1 MiB)


def _build(Ls):
    """Build the per-core Bass kernel. Ls: 16 request lengths, sorted desc."""
    nblk = [l // P + 1 for l in Ls]          # t-blocks incl. the new token
    r = [l % P for l in Ls]                  # new-token row within tail block
    vt = [rr + 1 for rr in r]                # valid rows in tail block
    jmax = max(nblk)
    # tile j is touched by requests [0, nbj[j]) (lengths sorted descending)
    nbj = [sum(1 for b in range(B) if nblk[b] > j) for j in range(jmax)]

    nc = bass.Bass()
    x_d = nc.dram_tensor("x", [B, D], F32, kind="ExternalInput")
    lnw_d = nc.dram_tensor("lnw", [P, NKC], F32, kind="ExternalInput")
    wq_d = nc.dram_tensor("wq", [D, NHL * HD], BF16, kind="ExternalInput")
    wk_d = nc.dram_tensor("wk", [D, NHL * HD], BF16, kind="ExternalInput")
    wv_d = nc.dram_tensor("wv", [D, NHL * HD], BF16, kind="ExternalInput")
    wo_d = nc.dram_tensor("wo", [NHL * HD, D], BF16, kind="ExternalInput")
    ktc_d = nc.dram_tensor("ktc", [B, NHL, HD, T], BF16, kind="ExternalInput")
    vc_d = nc.dram_tensor("vc", [B, T, NHL * HD], BF16, kind="ExternalInput")
    out_d = nc.dram_tensor("out", [B, D], F32, kind="ExternalOutput")

    with tile.TileContext(nc) as tc:
        with (
            tc.tile_pool(name="const", bufs=1) as const_pool,
            tc.tile_pool(name="persist", bufs=1) as persist,
            tc.tile_pool(name="pcols", bufs=1) as p_pool,
        ):
            identity = const_pool.tile([P, P], F32, tag="identity")
            make_identity(nc, identity[:])
            ones_col = const_pool.tile([P, 1], BF16, tag="ones")
            nc.gpsimd.memset(ones_col[:], 1.0)
            identity_bf = const_pool.tile([P, P], BF16, tag="idbf")
            make_identity(nc, identity_bf[:])
            zrow = const_pool.tile([1, 512], F32, tag="zrow")
            nc.gpsimd.memset(zrow[:], 0.0)
            lnw_sb = const_pool.tile([P, NKC], F32, tag="lnw")
            nc.sync.dma_start(lnw_sb[:], lnw_d[:, :])

            # ---------------- phase 1: RMSNorm + projections ----------------
            qt_sb = persist.tile([P, B * NHL], BF16, tag="qt")  # col h*16+b
            kt_sb = persist.tile([P, B * NHL], BF16, tag="kt")  # col h*16+b
            v_sb = persist.tile([B, NHL * HD], BF16, tag="vsb")
            xnt_sb = persist.tile([P, NKC * B], BF16, tag="xnt")  # chunk kc at cols kc*16

            # ---------------- score PSUM setup ----------------
            zatt_cm = tc.tile_pool(name="zatt", bufs=1, space="PSUM")
            zatt_pool = zatt_cm.__enter__()
            sc_cm = tc.tile_pool(name="sc", bufs=1, space="PSUM")
            sc_pool = sc_cm.__enter__()
            n_sc_banks = (jmax + 7) // 8
            sc_ps = [
                sc_pool.tile([P, 512], F32, tag=f"sc{i}", name=f"sc{i}")
                for i in range(n_sc_banks)
            ]
            z_ps = zatt_pool.tile([1, B * NHL], F32, tag="z")
            attn_ps = zatt_pool.tile([P, B * NHL], F32, tag="attn")

            def sc_slice(j, c0, c1, p0, p1):
                return sc_ps[j // 8][p0:p1, (j % 8) * 64 + c0 : (j % 8) * 64 + c1]

            # memset all score banks to -1e30 (invalid rows stay masked)
            for t_ in sc_ps:
                nc.vector.memset(t_[:], -1.0e30)
            nc.tensor.matmul(
                z_ps[:], zrow[0:1, 0:1], zrow[0:1, : B * NHL],
                start=True, stop=True, skip_group_check=True,
            )
            nc.tensor.matmul(
                attn_ps[:], zrow[0:1, :P], zrow[0:1, : B * NHL],
                start=True, stop=True, skip_group_check=True,
            )

            p1sb_cm = tc.tile_pool(name="p1sb", bufs=1)
            p1sb = p1sb_cm.__enter__()
            wpool_cm = tc.tile_pool(name="wpool", bufs=2)
            wpool = wpool_cm.__enter__()
            xntp_cm = tc.tile_pool(name="xntp", bufs=2, space="PSUM")
            xntp = xntp_cm.__enter__()
            qkv_cm = tc.tile_pool(name="qkvps", bufs=2, space="PSUM")
            qkvps = qkv_cm.__enter__()

            x_sb = p1sb.tile([B, D], F32, tag="x")
            nc.sync.dma_start(x_sb[:], x_d[:, :])
            # transpose RAW x (scaled by ln_w only); the 1/rms factor is a
            # per-request scalar folded into the q/k/v PSUM evacuations, so
            # the transposes don't wait on the reduction chain
            for kc in range(NKC):
                xnt_ps = xntp.tile([P, B], F32, tag="xntps")
                nc.tensor.transpose(
                    xnt_ps[:],
                    x_sb[:, kc * P : (kc + 1) * P],
                    identity[:B, :B],
                )
                nc.scalar.mul(
                    xnt_sb[:, kc * B : (kc + 1) * B], xnt_ps[:],
                    lnw_sb[:, kc : kc + 1],
                )
            xsq = p1sb.tile([B, D], F32, tag="xsq")
            ssq = p1sb.tile([B, 1], F32, tag="ssq")
            nc.scalar.activation(
                xsq[:], x_sb[:], mybir.ActivationFunctionType.Square,
                accum_out=ssq[:],
            )
            ssq2 = p1sb.tile([B, 1], F32, tag="ssq2")
            nc.vector.tensor_scalar_add(ssq2[:], ssq[:], EPS * D)
            std = p1sb.tile([B, 1], F32, tag="std")
            nc.scalar.activation(
                std[:], ssq2[:], mybir.ActivationFunctionType.Sqrt,
                scale=1.0 / D,
            )
            rstd = p1sb.tile([B, 1], F32, tag="rstd")
            nc.vector.reciprocal(rstd[:], std[:])

            # natural-form projections [16, 512]; each accumulator is alone in
            # its bank so plain start/stop group semantics are safe
            q_ps = qkvps.tile([B, NHL * HD], F32, tag="qkv", name="qps")
            WGRP = 8

            def w_load(pool_tag, dram):
                tiles = []
                for g in range(NKC // WGRP):
                    w_big = wpool.tile(
                        [P, WGRP * NHL * HD], BF16, tag=pool_tag,
                        name=f"{pool_tag}{g}",
                    )
                    srcw = dram[
                        g * WGRP * P : (g + 1) * WGRP * P, :
                    ].rearrange("(kc p) n -> p kc n", p=P)
                    nc.sync.dma_start(
                        w_big[:].rearrange("p (kc n) -> p kc n", n=NHL * HD),
                        srcw,
                    )
                    tiles.append(w_big)
                return tiles

            def proj(w_tiles, acc_ps):
                for kc in range(NKC):
                    wslice = w_tiles[kc // WGRP][
                        :, (kc % WGRP) * NHL * HD : (kc % WGRP + 1) * NHL * HD
                    ]
                    nc.tensor.matmul(
                        acc_ps[:], xnt_sb[:, kc * B : (kc + 1) * B], wslice,
                        start=(kc == 0), stop=(kc == NKC - 1),
                    )

            def transpose4(nat_sb, dst_sb):
                for h in range(NHL):
                    t_ps = xntp.tile([P, B], F32, tag="xntps", name=f"tp{h}")
                    nc.tensor.transpose(
                        t_ps[:], nat_sb[:, h * HD : (h + 1) * HD],
                        identity[:B, :B],
                    )
                    nc.scalar.copy(dst_sb[:, h * B : (h + 1) * B], t_ps[:])

            wq_tiles = w_load("wq", wq_d)
            proj(wq_tiles, q_ps)
            q_nat = p1sb.tile([B, NHL * HD], F32, tag="qnat")
            nc.scalar.mul(q_nat[:], q_ps[:], rstd[:])
            transpose4(q_nat, qt_sb)

            # ---------------- pass K main (full blocks; needs only qt) ------
            # K is host-pre-transposed: ktc[b, h, d, t]. One DMA per request
            # covers all 4 heads' full cache blocks; scores read lhsT slices
            # straight out of the tile (no PE transpose, no evacuation copy).
            TW = T
            with tc.tile_pool(name="ktbig", bufs=3) as ktb_pool:
                for b in range(B):
                    L = Ls[b]
                    f = L // P
                    if f > 0:
                        ktb = ktb_pool.tile([P, NHL * TW], BF16, tag="ktb")
                        nc.sync.dma_start(
                            ktb[:].rearrange("p (h t) -> p h t", t=TW)[
                                :, :, : f * P
                            ],
                            ktc_d[b, :, :, : f * P].rearrange("h d t -> d h t"),
                        )
                        for j in range(f):
                            for h in range(NHL):
                                col = b * NHL + h
                                nc.tensor.matmul(
                                    sc_slice(j, col, col + 1, 0, P),
                                    ktb[:, h * TW + j * P : h * TW + (j + 1) * P],
                                    qt_sb[:, h * B + b : h * B + b + 1],
                                    start=True, stop=True,
                                )

            # k projection (weights prefetched during the scores above)
            k_ps = qkvps.tile([B, NHL * HD], F32, tag="qkv", name="kps")
            wk_tiles = w_load("wk", wk_d)
            proj(wk_tiles, k_ps)
            k_nat = p1sb.tile([B, NHL * HD], F32, tag="knat")
            nc.scalar.mul(k_nat[:], k_ps[:], rstd[:])
            transpose4(k_nat, kt_sb)

            # ---------------- pass K tails (need kt for the new token) ------
            with tc.tile_pool(name="kttail", bufs=4) as ktt_pool:
                for b in range(B):
                    L = Ls[b]
                    jt = L // P
                    ktt = ktt_pool.tile([P, NHL * P], BF16, tag="ktt")
                    if r[b] > 0:
                        nc.sync.dma_start(
                            ktt[:].rearrange("p (h t) -> p h t", t=P)[
                                :, :, : r[b]
                            ],
                            ktc_d[
                                b, :, :, jt * P : jt * P + r[b]
                            ].rearrange("h d t -> d h t"),
                        )
                    for h in range(NHL):
                        nc.vector.tensor_copy(
                            ktt[:, h * P + r[b] : h * P + r[b] + 1],
                            kt_sb[:, h * B + b : h * B + b + 1],
                        )
                        col = b * NHL + h
                        nc.tensor.matmul(
                            sc_slice(jt, col, col + 1, 0, vt[b]),
                            ktt[:, h * P : h * P + vt[b]],
                            qt_sb[:, h * B + b : h * B + b + 1],
                            start=True, stop=True,
                        )

            # v projection
            v_ps = qkvps.tile([B, NHL * HD], F32, tag="qkv", name="vps")
            wv_tiles = w_load("wv", wv_d)
            proj(wv_tiles, v_ps)
            nc.scalar.mul(v_sb[:], v_ps[:], rstd[:])

            qkv_cm.__exit__(None, None, None)
            xntp_cm.__exit__(None, None, None)
            wpool_cm.__exit__(None, None, None)
            p1sb_cm.__exit__(None, None, None)

            # ---------------- softmax (no max-sub) ----------------
            p_cols = []
            for j in range(jmax):
                pc = p_pool.tile([P, B * NHL], BF16, tag=f"p{j}", name=f"p{j}")
                nc.scalar.activation(
                    pc[:, : NHL * nbj[j]],
                    sc_slice(j, 0, NHL * nbj[j], 0, P),
                    mybir.ActivationFunctionType.Exp,
                    scale=SCALE,
                )
                p_cols.append(pc)
            for j in range(jmax):
                nc.tensor.matmul(
                    z_ps[0:1, : NHL * nbj[j]],
                    ones_col[:],
                    p_cols[j][:, : NHL * nbj[j]],
                    start=False, stop=(j == jmax - 1),
                    skip_group_check=True,
                )
            invz_row = persist.tile([1, B * NHL], F32, tag="invzr")
            nc.vector.reciprocal(invz_row[:], z_ps[:])
            # bounce through DRAM, reordering (b,h) -> (h,b) on the second hop
            # so the result is a per-partition column matching attn's h*B+b
            # column order (free dims can't become partitions inside SBUF)
            invz_dram = nc.dram_tensor("invz_scratch", [1, B * NHL], F32)
            nc.gpsimd.dma_start(invz_dram[:, :], invz_row[:])
            invz_col = persist.tile([NHL * B, 1], F32, tag="invzc")
            for h in range(NHL):
                nc.gpsimd.dma_start(
                    invz_col[h * B : (h + 1) * B, :],
                    invz_dram.rearrange("o (b h) -> (o b) h", h=NHL)[:, h : h + 1],
                )

            # ---------------- pass V: attn = p @ V ----------------
            with tc.tile_pool(name="vpool", bufs=4) as vpool:
                for b in range(B):
                    L = Ls[b]
                    ngrp = (nblk[b] + JG - 1) // JG
                    for jg in range(ngrp):
                        rows_g = max(0, min(JG * P, L - jg * JG * P))
                        q128, rem = divmod(rows_g, P)
                        v_tile = vpool.tile([P, JG * NHL * HD], BF16, tag="v")
                        if q128 > 0:
                            src = vc_d[
                                b, jg * JG * P : jg * JG * P + q128 * P, :
                            ].rearrange("(jj p) d -> p jj d", p=P)
                            nc.sync.dma_start(
                                v_tile[:, : q128 * NHL * HD].rearrange(
                                    "p (jj d) -> p jj d", d=NHL * HD
                                ),
                                src,
                            )
                        if rem > 0:
                            nc.sync.dma_start(
                                v_tile[
                                    :rem, q128 * NHL * HD : (q128 + 1) * NHL * HD
                                ],
                                vc_d[
                                    b,
                                    jg * JG * P + q128 * P : jg * JG * P + rows_g,
                                    :,
                                ],
                            )
                        for jj in range(JG):
                            j = jg * JG + jj
                            if j >= nblk[b]:
                                break
                            tail = j == nblk[b] - 1
                            m = vt[b] if tail else P
                            if tail:
                                # splice the new token's v row in
                                nc.gpsimd.dma_start(
                                    v_tile[
                                        r[b] : r[b] + 1,
                                        jj * NHL * HD : (jj + 1) * NHL * HD,
                                    ],
                                    v_sb[b : b + 1, :],
                                )
                            for h in range(NHL):
                                col = b * NHL + h
                                # attn columns are head-major so o_proj's lhsT
                                # per head is a contiguous [128, 16] slice
                                nc.tensor.matmul(
                                    attn_ps[:, h * B + b : h * B + b + 1],
                                    v_tile[
                                        :m,
                                        jj * NHL * HD + h * HD : jj * NHL * HD + (h + 1) * HD,
                                    ],
                                    p_cols[j][:m, col : col + 1],
                                    start=False, stop=tail,
                                    skip_group_check=True,
                                )

            attn_sb = persist.tile([P, B * NHL], BF16, tag="attnsb")
            nc.scalar.copy(attn_sb[:], attn_ps[:])
            sc_cm.__exit__(None, None, None)

            # ---------------- o_proj partial with 1/Z ----------------
            # scale attn by 1/Z in its transposed domain, where (h,b) is the
            # partition index and 1/Z is a plain per-partition scalar; then
            # o_proj accumulates the 4 heads in PSUM (no per-head epilogue)
            out_sb = persist.tile([B, D], F32, tag="outsb")
            with (
                tc.tile_pool(name="wopool", bufs=1) as wopool,
                tc.tile_pool(name="ops", bufs=4, space="PSUM") as o_ps_pool,
            ):
                tr1 = o_ps_pool.tile([B * NHL, P], BF16, tag="tr1", bufs=1)
                nc.tensor.transpose(tr1[:], attn_sb[:], identity_bf[:])
                attn_n = persist.tile([B * NHL, P], BF16, tag="attnn")
                nc.scalar.mul(attn_n[:], tr1[:], invz_col[:])
                tr2 = o_ps_pool.tile([P, B * NHL], BF16, tag="tr2", bufs=1)
                nc.tensor.transpose(
                    tr2[:], attn_n[:], identity_bf[: B * NHL, : B * NHL]
                )
                attn_s = persist.tile([P, B * NHL], BF16, tag="attns")
                nc.vector.tensor_copy(attn_s[:], tr2[:])

                NCH = D // 512
                wo_tiles = []
                for h in range(NHL):
                    wo_sb = wopool.tile([P, D], BF16, tag=f"wo{h}", name=f"wo{h}")
                    nc.scalar.dma_start(wo_sb[:], wo_d[h * HD : (h + 1) * HD, :])
                    wo_tiles.append(wo_sb)
                for nch in range(NCH):
                    o_ps = o_ps_pool.tile([B, 512], F32, tag="ops", bufs=2)
                    for h in range(NHL):
                        nc.tensor.matmul(
                            o_ps[:],
                            attn_s[:, h * B : (h + 1) * B],
                            wo_tiles[h][:, nch * 512 : (nch + 1) * 512],
                            start=(h == 0), stop=(h == NHL - 1),
                        )
                    nc.scalar.copy(out_sb[:, nch * 512 : (nch + 1) * 512], o_ps[:])
            nc.sync.dma_start(out_d[:, :], out_sb[:])
            zatt_cm.__exit__(None, None, None)

    _split_excess_waits(nc)
    return nc


def _prep_inputs(x, ln_w, Wq, Wk, Wv, Wo, K_cache, V_cache, cache_lens):
    x = np.asarray(x, np.float32).reshape(B, D)
    ln_w = np.asarray(ln_w, np.float32)
    cache_lens = np.asarray(cache_lens, np.int32)
    perm = np.argsort(-cache_lens, kind="stable")
    Ls = [int(cache_lens[p]) for p in perm]
    lnw2d = np.ascontiguousarray(ln_w.reshape(NKC, P).T)
    x_s = np.ascontiguousarray(x[perm])
    K4 = np.asarray(K_cache, np.float32).reshape(B, T, H, HD)
    V4 = np.asarray(V_cache, np.float32).reshape(B, T, H, HD)
    in_maps = []
    for c in range(NCORES):
        h0 = c * NHL
        in_maps.append(
            {
                "x": x_s,
                "lnw": lnw2d,
                "wq": np.ascontiguousarray(
                    np.asarray(Wq, np.float32)[:, h0 * HD : (h0 + NHL) * HD]
                ).astype(BF16_NP),
                "wk": np.ascontiguousarray(
                    np.asarray(Wk, np.float32)[:, h0 * HD : (h0 + NHL) * HD]
                ).astype(BF16_NP),
                "wv": np.ascontiguousarray(
                    np.asarray(Wv, np.float32)[:, h0 * HD : (h0 + NHL) * HD]
                ).astype(BF16_NP),
                "wo": np.ascontiguousarray(
                    np.asarray(Wo, np.float32)[h0 * HD : (h0 + NHL) * HD, :]
                ).astype(BF16_NP),
                "ktc": np.ascontiguousarray(
                    K4[perm][:, :, h0 : h0 + NHL, :].transpose(0, 2, 3, 1)
                ).astype(BF16_NP),
                "vc": np.ascontiguousarray(
                    V4[perm][:, :, h0 : h0 + NHL, :]
                ).reshape(B, T, NHL * HD).astype(BF16_NP),
            }
        )
    return in_maps, Ls, perm, x_s


def _run(x, ln_w, Wq, Wk, Wv, Wo, K_cache, V_cache, cache_lens, trace=False):
    in_maps, Ls, perm, x_s = _prep_inputs(
        x, ln_w, Wq, Wk, Wv, Wo, K_cache, V_cache, cache_lens
    )
    nc = _build(Ls)
    # the axon-proxied runtime occasionally hits a transient
    # NRT_EXEC_UNIT_UNRECOVERABLE; retry a couple of times
    last_exc = None
    for _attempt in range(3):
        try:
            res = run_bass_kernel_spmd(
                nc, in_maps, core_ids=list(range(NCORES)), trace=trace
            )
            break
        except Exception as e:  # noqa: BLE001
            last_exc = e
            import time as _time

            _time.sleep(2.0)
    else:
        raise last_exc
    partial = np.zeros((B, D), np.float32)
    for c in range(NCORES):
        partial += res.results[c]["out"]
    out_sorted = x_s + partial
    out = np.empty((B, D), np.float32)
    out[perm] = out_sorted
    return out.reshape(B, 1, D), res


def kernel(x, ln_w, Wq, Wk, Wv, Wo, K_cache, V_cache, cache_lens):
    out, _ = _run(x, ln_w, Wq, Wk, Wv, Wo, K_cache, V_cache, cache_lens)
    return out
